# revision 8
# baseline (speedup 1.0000x reference)
"""EdgeModel GNN message-passing kernel for 8 Trainium2 NeuronCores.

Reference computation (per edge e with endpoints row[e], col[e]):
    e1 = tanh(edge_attr @ W1 + b1)                         # [E, 128]
    h  = relu(BN(concat(x[row], x[col], e1) @ W2 + b2))    # [E, 128]
    y  = relu(h @ W3 + b3)                                 # [E, 128]

Strategy:
  - Data-parallel over edges: each of the 8 cores owns E/8 edges (padded to a
    multiple of 2048). x and all weights are replicated per core.
  - BN (eval mode) is folded into W2/b2 on the host.
  - Gathers of x[row], x[col] are done on-device with large indirect DMAs
    (2048 rows / 1 MB per instruction) to amortize SWDGE fixed cost.
  - Compute pipeline is feature-major (features on partitions, edges on the
    free dim) so all weight matrices are static lhsT operands:
        eaT  = PE-transpose(ea tile)              [32*4, 128e]
        e1T  = tanh(W1^T @ eaT + b1)              [128f, 512e]
        xrT/xcT = PE-transpose(gathered tiles)    [128f, 512e]
        hT   = relu(W2a'^T xrT + W2b'^T xcT + W2c'^T e1T + b2')
        yT   = relu(W3^T hT + b3)
    yT stored as [128, Ep] per core; host untransposes/reorders.

Edge slot mapping (must match between host index/ea prep and device layout):
  group g (512 edges), block c in 0..3, partition p in 0..127
  free position k = 128*c + p  <->  edge  512*g + 4*p + c
"""

import numpy as np

NC = 8
N_NODES = 100000
E_TOTAL = 500000
NF = 128  # node features
IF = 32   # edge in features
OF = 128  # hidden / out features
BN_EPS = 1e-5

GROUP = 512           # edges per matmul group
SG_GROUPS = 4         # groups per supergroup (gather granularity)
SG = GROUP * SG_GROUPS  # 2048 edges gathered per indirect DMA

E_PER_CORE = (E_TOTAL + NC - 1) // NC            # 62500
N_SG = (E_PER_CORE + SG - 1) // SG               # 31
EP = N_SG * SG                                   # 63488 padded edges/core
N_GROUPS = EP // GROUP                           # 124
N_BLOCKS = EP // 128                             # 496

# matmul dtype knobs
MM_DTYPE_BIG = "float32r"   # K=128 matmuls (N=512): fp32r = 1 cyc/row
MM_DTYPE_EA = "float32"     # MM1 (K=32, N=128)

_PROGRAM_CACHE = {}


def _build_program(n_nodes, ep, n_sg):
    """Build the per-core Bass/Tile program (same program on all 8 cores)."""
    import concourse.bacc as bacc
    import concourse.bass as bass
    import concourse.mybir as mybir
    import concourse.tile as tile
    from concourse.masks import make_identity

    f32 = mybir.dt.float32
    i32 = mybir.dt.int32
    dt_big = getattr(mybir.dt, MM_DTYPE_BIG)
    dt_ea = getattr(mybir.dt, MM_DTYPE_EA)

    n_groups = n_sg * SG_GROUPS
    n_blocks = ep // 128

    nc = bacc.Bacc(
        "TRN2",
        target_bir_lowering=False,
        debug=False,
        enable_asserts=False,
        num_devices=NC,
    )

    x_d = nc.dram_tensor("x", [n_nodes, NF], f32, kind="ExternalInput").ap()
    ea_d = nc.dram_tensor("ea", [ep, IF], f32, kind="ExternalInput").ap()
    ridx_d = nc.dram_tensor("ridx", [128, n_blocks], i32, kind="ExternalInput").ap()
    cidx_d = nc.dram_tensor("cidx", [128, n_blocks], i32, kind="ExternalInput").ap()
    w1_d = nc.dram_tensor("w1", [IF, OF], dt_big, kind="ExternalInput").ap()
    w2a_d = nc.dram_tensor("w2a", [NF, OF], dt_big, kind="ExternalInput").ap()
    w2b_d = nc.dram_tensor("w2b", [NF, OF], dt_big, kind="ExternalInput").ap()
    w2c_d = nc.dram_tensor("w2c", [OF, OF], dt_big, kind="ExternalInput").ap()
    w3_d = nc.dram_tensor("w3", [OF, OF], dt_big, kind="ExternalInput").ap()
    b1_d = nc.dram_tensor("b1", [128, 1], f32, kind="ExternalInput").ap()
    b2_d = nc.dram_tensor("b2", [128, 1], f32, kind="ExternalInput").ap()
    b3_d = nc.dram_tensor("b3", [128, 1], f32, kind="ExternalInput").ap()
    yt_d = nc.dram_tensor("yt", [128, ep], f32, kind="ExternalOutput").ap()

    # ea viewed so that partition p of group-g tile holds edges 512g+4p+{0..3}
    ea_view = ea_d.rearrange("(g p t) f -> g p (t f)", p=128, t=SG_GROUPS)

    Tanh = mybir.ActivationFunctionType.Tanh
    Relu = mybir.ActivationFunctionType.Relu

    with tile.TileContext(nc) as tc:
        with (
            tc.tile_pool(name="const", bufs=1) as cpool,
            tc.tile_pool(name="idx", bufs=1) as ipool,
            tc.tile_pool(name="gather", bufs=2) as gpool,
            tc.tile_pool(name="eain", bufs=3) as eapool,
            tc.tile_pool(name="feat", bufs=2) as fpool,
            tc.tile_pool(name="out", bufs=3) as opool,
            tc.tile_pool(name="ps_small", bufs=1, space="PSUM") as ps_small,
            tc.tile_pool(name="ps_e", bufs=2, space="PSUM") as ps_e,
            tc.tile_pool(name="ps_x", bufs=1, space="PSUM") as ps_x,
            tc.tile_pool(name="ps_h", bufs=2, space="PSUM") as ps_h,
            tc.tile_pool(name="ps_y", bufs=1, space="PSUM") as ps_y,
        ):
            ident = cpool.tile([128, 128], f32, tag="ident")
            make_identity(nc, ident[:])

            w1_sb = cpool.tile([IF, OF], dt_big, tag="w1")
            nc.sync.dma_start(w1_sb[:], w1_d[:, :])
            w2a_sb = cpool.tile([NF, OF], dt_big, tag="w2a")
            nc.sync.dma_start(w2a_sb[:], w2a_d[:, :])
            w2b_sb = cpool.tile([NF, OF], dt_big, tag="w2b")
            nc.sync.dma_start(w2b_sb[:], w2b_d[:, :])
            w2c_sb = cpool.tile([OF, OF], dt_big, tag="w2c")
            nc.sync.dma_start(w2c_sb[:], w2c_d[:, :])
            w3_sb = cpool.tile([OF, OF], dt_big, tag="w3")
            nc.sync.dma_start(w3_sb[:], w3_d[:, :])
            b1_sb = cpool.tile([128, 1], f32, tag="b1")
            nc.sync.dma_start(b1_sb[:], b1_d[:, :])
            b2_sb = cpool.tile([128, 1], f32, tag="b2")
            nc.sync.dma_start(b2_sb[:], b2_d[:, :])
            b3_sb = cpool.tile([128, 1], f32, tag="b3")
            nc.sync.dma_start(b3_sb[:], b3_d[:, :])

            ridx_sb = ipool.tile([128, n_blocks], i32, tag="ridx")
            nc.sync.dma_start(ridx_sb[:], ridx_d[:, :])
            cidx_sb = ipool.tile([128, n_blocks], i32, tag="cidx")
            nc.sync.dma_start(cidx_sb[:], cidx_d[:, :])

            for s in range(n_sg):
                for gi in range(SG_GROUPS):
                    g = SG_GROUPS * s + gi

                    # gather 512 node rows per endpoint for this group
                    # (HW indirect DMA: one offset per partition per instr)
                    xr_sb = gpool.tile([128, 4 * NF], f32, tag="xr")
                    xc_sb = gpool.tile([128, 4 * NF], f32, tag="xc")
                    for c in range(4):
                        B = 4 * g + c
                        nc.gpsimd.indirect_dma_start(
                            out=xr_sb[:, 128 * c : 128 * (c + 1)],
                            out_offset=None,
                            in_=x_d[:, :],
                            in_offset=bass.IndirectOffsetOnAxis(
                                ap=ridx_sb[:, B : B + 1], axis=0
                            ),
                        )
                        nc.gpsimd.indirect_dma_start(
                            out=xc_sb[:, 128 * c : 128 * (c + 1)],
                            out_offset=None,
                            in_=x_d[:, :],
                            in_offset=bass.IndirectOffsetOnAxis(
                                ap=cidx_sb[:, B : B + 1], axis=0
                            ),
                        )

                    # --- e1 = tanh(ea @ W1 + b1), feature-major ---
                    ea_sb = eapool.tile([128, 128], f32, tag="ea")
                    nc.sync.dma_start(ea_sb[:], ea_view[g])
                    eaT_ps = ps_small.tile([IF, GROUP], f32, tag="eaT")
                    for t in range(4):
                        nc.tensor.transpose(
                            eaT_ps[:, 128 * t : 128 * (t + 1)],
                            ea_sb[:, 32 * t : 32 * (t + 1)],
                            ident[:],
                        )
                    eaT_sb = eapool.tile([IF, GROUP], dt_big, tag="eaT_sb")
                    nc.vector.tensor_copy(eaT_sb[:], eaT_ps[:])

                    e_ps = ps_e.tile([128, GROUP], f32, tag="e")
                    nc.tensor.matmul(
                        e_ps[:], lhsT=w1_sb[:], rhs=eaT_sb[:], start=True, stop=True
                    )
                    eT_sb = fpool.tile([128, GROUP], dt_big, tag="eT")
                    nc.scalar.activation(eT_sb[:], e_ps[:], Tanh, bias=b1_sb[:, :1])

                    # --- transpose gathered x rows to feature-major ---
                    xrT_ps = ps_x.tile([128, GROUP], f32, tag="xrT")
                    xcT_ps = ps_x.tile([128, GROUP], f32, tag="xcT")
                    for c in range(4):
                        nc.tensor.transpose(
                            xrT_ps[:, 128 * c : 128 * (c + 1)],
                            xr_sb[:, 128 * c : 128 * (c + 1)],
                            ident[:],
                        )
                        nc.tensor.transpose(
                            xcT_ps[:, 128 * c : 128 * (c + 1)],
                            xc_sb[:, 128 * c : 128 * (c + 1)],
                            ident[:],
                        )
                    xrT_sb = fpool.tile([128, GROUP], dt_big, tag="xrT_sb")
                    nc.vector.tensor_copy(xrT_sb[:], xrT_ps[:])
                    xcT_sb = fpool.tile([128, GROUP], dt_big, tag="xcT_sb")
                    nc.vector.tensor_copy(xcT_sb[:], xcT_ps[:])

                    # --- h = relu(xr@W2a' + xc@W2b' + e1@W2c' + b2') ---
                    h_ps = ps_h.tile([128, GROUP], f32, tag="h")
                    nc.tensor.matmul(
                        h_ps[:],
                        lhsT=w2a_sb[:],
                        rhs=xrT_sb[:],
                        start=True,
                        stop=False,
                    )
                    nc.tensor.matmul(
                        h_ps[:],
                        lhsT=w2b_sb[:],
                        rhs=xcT_sb[:],
                        start=False,
                        stop=False,
                    )
                    nc.tensor.matmul(
                        h_ps[:],
                        lhsT=w2c_sb[:],
                        rhs=eT_sb[:],
                        start=False,
                        stop=True,
                    )
                    hT_sb = fpool.tile([128, GROUP], dt_big, tag="hT")
                    nc.scalar.activation(hT_sb[:], h_ps[:], Relu, bias=b2_sb[:, :1])

                    # --- y = relu(h @ W3 + b3) ---
                    y_ps = ps_y.tile([128, GROUP], f32, tag="y")
                    nc.tensor.matmul(
                        y_ps[:],
                        lhsT=w3_sb[:],
                        rhs=hT_sb[:],
                        start=True,
                        stop=True,
                    )
                    yT_sb = opool.tile([128, GROUP], f32, tag="yT")
                    nc.vector.tensor_scalar(
                        yT_sb[:],
                        y_ps[:],
                        b3_sb[:, :1],
                        0.0,
                        op0=mybir.AluOpType.add,
                        op1=mybir.AluOpType.max,
                    )
                    nc.sync.dma_start(yt_d[:, GROUP * g : GROUP * (g + 1)], yT_sb[:])

    nc.compile()
    return nc


def _get_program(n_nodes=N_NODES, ep=EP, n_sg=N_SG):
    key = (n_nodes, ep, n_sg)
    if key not in _PROGRAM_CACHE:
        _PROGRAM_CACHE[key] = _build_program(n_nodes, ep, n_sg)
    return _PROGRAM_CACHE[key]


def _fold_weights(W1, b1, W2, b2, bn_gamma, bn_beta, bn_mean, bn_var, W3, b3):
    s = bn_gamma / np.sqrt(bn_var + BN_EPS)
    W2f = (W2 * s[None, :]).astype(np.float32)
    b2f = ((b2 - bn_mean) * s + bn_beta).astype(np.float32)
    w2a = np.ascontiguousarray(W2f[:NF])
    w2b = np.ascontiguousarray(W2f[NF : 2 * NF])
    w2c = np.ascontiguousarray(W2f[2 * NF :])
    return (
        np.ascontiguousarray(np.asarray(W1, np.float32)),
        w2a,
        w2b,
        w2c,
        np.ascontiguousarray(np.asarray(W3, np.float32)),
        np.asarray(b1, np.float32).reshape(128, 1).copy(),
        b2f.reshape(128, 1).copy(),
        np.asarray(b3, np.float32).reshape(128, 1).copy(),
    )


def _block_indices(idx_core, ep):
    """idx_core: [ep] node indices in edge order. Returns [128, ep//128] int32
    where out[p, B] = idx_core[512*(B//4) + 4*p + (B%4)]."""
    n_groups = ep // GROUP
    a = idx_core.reshape(n_groups, 128, 4)          # [g, p, c] -> edge 512g+4p+c
    a = a.transpose(1, 0, 2).reshape(128, n_groups * 4)  # [p, (g,c)] = [p, B]
    return np.ascontiguousarray(a.astype(np.int32))


def _prepare_in_maps(inputs):
    x = np.ascontiguousarray(np.asarray(inputs["x"], np.float32))
    edge_index = np.asarray(inputs["edge_index"])
    edge_attr = np.asarray(inputs["edge_attr"], np.float32)
    folded = _fold_weights(
        inputs["W1"], inputs["b1"], inputs["W2"], inputs["b2"],
        inputs["bn_gamma"], inputs["bn_beta"], inputs["bn_mean"],
        inputs["bn_var"], inputs["W3"], inputs["b3"],
    )
    w1rep, w2a, w2b, w2c, w3, b1t, b2t, b3t = folded

    E = edge_index.shape[1]
    row = np.asarray(edge_index[0], np.int64)
    col = np.asarray(edge_index[1], np.int64)

    in_maps = []
    for c in range(NC):
        lo = c * E_PER_CORE
        hi = min(E, lo + E_PER_CORE)
        n = hi - lo
        ea_c = np.zeros((EP, IF), np.float32)
        ea_c[:n] = edge_attr[lo:hi]
        row_c = np.zeros(EP, np.int64)
        row_c[:n] = row[lo:hi]
        col_c = np.zeros(EP, np.int64)
        col_c[:n] = col[lo:hi]
        in_maps.append(
            {
                "x": x,
                "ea": ea_c,
                "ridx": _block_indices(row_c, EP),
                "cidx": _block_indices(col_c, EP),
                "w1": w1rep,
                "w2a": w2a,
                "w2b": w2b,
                "w2c": w2c,
                "w3": w3,
                "b1": b1t,
                "b2": b2t,
                "b3": b3t,
            }
        )
    return in_maps, E


def _postprocess(results, E):
    out = np.empty((E, OF), np.float32)
    for c in range(NC):
        lo = c * E_PER_CORE
        hi = min(E, lo + E_PER_CORE)
        yt = results[c]["yt"]  # [128, EP]
        # free slot 512g + 128c4 + p  <->  edge 512g + 4p + c4
        y = yt.reshape(OF, N_GROUPS, 4, 128).transpose(1, 3, 2, 0).reshape(EP, OF)
        out[lo:hi] = y[: hi - lo]
    return out


def kernel(**inputs):
    from concourse import bass_utils

    in_maps, E = _prepare_in_maps(inputs)
    nc = _get_program()
    res = bass_utils.run_bass_kernel_spmd(nc, in_maps, core_ids=list(range(NC)))
    return _postprocess(res.results, E)


# revision 16
# speedup vs baseline: 1.9364x; 1.9364x over previous
"""EdgeModel GNN message-passing kernel for 8 Trainium2 NeuronCores.

Reference computation (per edge e with endpoints row[e], col[e]):
    e1 = tanh(edge_attr @ W1 + b1)                         # [E, 128]
    h  = relu(BN(concat(x[row], x[col], e1) @ W2 + b2))    # [E, 128]
    y  = relu(h @ W3 + b3)                                 # [E, 128]

Strategy (v3):
  - Data-parallel over edges: each of the 8 cores owns E/8 edges; x and all
    weights replicated per core. BN (eval) folded into W2/b2 on host.
  - Each core gets its OWN program (data-shaped), dispatched asynchronously
    to its device; this allows data-dependent instruction structure.
  - Per core, edges are sorted by row index. The x[row] stream is produced
    WITHOUT per-row DMA descriptors (Q7 SWDGE descriptor generation is the
    bottleneck at ~8 ns/row): x is streamed sequentially in 128-row chunks,
    and each 512-slot group is expanded from the few chunks its (sorted)
    rows fall into with one-hot selection matmuls:
        xrT[:, a:b] = x_chunk^T @ Sel[:, a:b]   (PE, feature-major output)
    Sel is built on-chip: PE rank-1 broadcast of (row%128) + DVE is_equal
    against a lane-index column.
  - x[col] (random order) still uses the Q7 indirect gather, one [128,1]
    offset instruction per 128 slots (~62.5k descriptors/core).
  - Compute pipeline is feature-major (features on partitions, edges on
    free dim); weights are static lhsT; K=128 matmuls in fp32r:
        e1T = tanh(W1^T @ eaT + b1);  hT = relu(W2'^T [xrT;xcT;e1T] + b2')
        yT  = relu(W3^T hT + b3) -> stored [128, EP]; host un-sorts.

Slot mapping: group g, free position k in [0,512) <-> slot 512g + k (the
host's row-sorted edge order). ea DRAM is pre-permuted so the contiguous
per-partition load gives ea_sb[p, (t,f)] = ea[slot 512g + 128t + p]; the
four [128,32]->[32,128] PE transposes then land eaT free-position-aligned.
xc gather block B=4g+c partition p serves slot 512g + 128c + p.
"""

import numpy as np

NC = 8
N_NODES = 100000
E_TOTAL = 500000
NF = 128
IF = 32
OF = 128
BN_EPS = 1e-5

GROUP = 512
E_PER_CORE = (E_TOTAL + NC - 1) // NC  # 62500

SUPER = 8                      # x chunks per streaming DMA (1024 rows)
X_PAD_ROWS = -(-N_NODES // (128 * SUPER)) * 128 * SUPER  # 100352
N_SUPERS = X_PAD_ROWS // (128 * SUPER)                   # 98

_PROGRAM_CACHE = {}


def _build_core_program(n_groups, segments, super_first_use):
    """One core's program.
    segments: per group, list of (chunk_id, a, b) half-open free ranges.
    super_first_use: per group, super-chunk ids to load before it."""
    import concourse.bacc as bacc
    import concourse.bass as bass
    import concourse.mybir as mybir
    import concourse.tile as tile
    from concourse.masks import make_identity

    f32 = mybir.dt.float32
    f32r = mybir.dt.float32r
    i32 = mybir.dt.int32
    bf16 = mybir.dt.bfloat16

    ep = n_groups * GROUP

    nc = bacc.Bacc(
        "TRN2",
        target_bir_lowering=False,
        debug=False,
        enable_asserts=False,
    )

    x_d = nc.dram_tensor("x", [X_PAD_ROWS, NF], f32, kind="ExternalInput").ap()
    ea_d = nc.dram_tensor("ea", [ep, IF], f32, kind="ExternalInput").ap()
    cidx_d = nc.dram_tensor("cidx", [128, ep // 128], i32, kind="ExternalInput").ap()
    rmod_d = nc.dram_tensor("rmod", [1, ep], bf16, kind="ExternalInput").ap()
    lane_d = nc.dram_tensor("lane", [128, 1], f32, kind="ExternalInput").ap()
    w1_d = nc.dram_tensor("w1", [IF, OF], f32r, kind="ExternalInput").ap()
    w2a_d = nc.dram_tensor("w2a", [NF, OF], f32r, kind="ExternalInput").ap()
    w2b_d = nc.dram_tensor("w2b", [NF, OF], f32r, kind="ExternalInput").ap()
    w2c_d = nc.dram_tensor("w2c", [OF, OF], f32r, kind="ExternalInput").ap()
    w3_d = nc.dram_tensor("w3", [OF, OF], f32r, kind="ExternalInput").ap()
    b1_d = nc.dram_tensor("b1", [128, 1], f32, kind="ExternalInput").ap()
    b2_d = nc.dram_tensor("b2", [128, 1], f32, kind="ExternalInput").ap()
    b3_d = nc.dram_tensor("b3", [128, 1], f32, kind="ExternalInput").ap()
    yt_d = nc.dram_tensor("yt", [128, ep], f32, kind="ExternalOutput").ap()

    ea_view = ea_d.rearrange("(g p t) f -> g p (t f)", p=128, t=4)
    # super-chunk s, partition p holds x rows {1024 s + 128 k + p}
    x_view = x_d.rearrange("(s k p) f -> s p k f", p=128, k=SUPER)

    Tanh = mybir.ActivationFunctionType.Tanh
    Relu = mybir.ActivationFunctionType.Relu

    with tile.TileContext(nc) as tc:
        with (
            tc.tile_pool(name="const", bufs=1) as cpool,
            tc.tile_pool(name="idx", bufs=1) as ipool,
            tc.tile_pool(name="xs", bufs=4) as xspool,
            tc.tile_pool(name="gather", bufs=2) as gpool,
            tc.tile_pool(name="eain", bufs=3) as eapool,
            tc.tile_pool(name="sel", bufs=2) as selpool,
            tc.tile_pool(name="feat", bufs=2) as fpool,
            tc.tile_pool(name="out", bufs=3) as opool,
            tc.tile_pool(name="ps_eaT", bufs=1, space="PSUM") as ps_eaT,
            tc.tile_pool(name="ps_bc", bufs=1, space="PSUM") as ps_bc,
            tc.tile_pool(name="ps_e", bufs=1, space="PSUM") as ps_e,
            tc.tile_pool(name="ps_x", bufs=1, space="PSUM") as ps_x,
            tc.tile_pool(name="ps_h", bufs=2, space="PSUM") as ps_h,
            tc.tile_pool(name="ps_y", bufs=1, space="PSUM") as ps_y,
        ):
            ident = cpool.tile([128, 128], f32, tag="ident")
            make_identity(nc, ident[:])
            ones_sb = cpool.tile([1, 128], bf16, tag="ones")
            nc.gpsimd.memset(ones_sb[:], 1.0)
            lane_sb = cpool.tile([128, 1], f32, tag="lane")
            nc.sync.dma_start(lane_sb[:], lane_d[:, :])

            w1_sb = cpool.tile([IF, OF], f32r, tag="w1")
            nc.sync.dma_start(w1_sb[:], w1_d[:, :])
            w2a_sb = cpool.tile([NF, OF], f32r, tag="w2a")
            nc.sync.dma_start(w2a_sb[:], w2a_d[:, :])
            w2b_sb = cpool.tile([NF, OF], f32r, tag="w2b")
            nc.sync.dma_start(w2b_sb[:], w2b_d[:, :])
            w2c_sb = cpool.tile([OF, OF], f32r, tag="w2c")
            nc.sync.dma_start(w2c_sb[:], w2c_d[:, :])
            w3_sb = cpool.tile([OF, OF], f32r, tag="w3")
            nc.sync.dma_start(w3_sb[:], w3_d[:, :])
            b1_sb = cpool.tile([128, 1], f32, tag="b1")
            nc.sync.dma_start(b1_sb[:], b1_d[:, :])
            b2_sb = cpool.tile([128, 1], f32, tag="b2")
            nc.sync.dma_start(b2_sb[:], b2_d[:, :])
            b3_sb = cpool.tile([128, 1], f32, tag="b3")
            nc.sync.dma_start(b3_sb[:], b3_d[:, :])

            cidx_sb = ipool.tile([128, ep // 128], i32, tag="cidx")
            nc.sync.dma_start(cidx_sb[:], cidx_d[:, :])

            super_tiles = {}
            for g in range(n_groups):
                for sc in super_first_use[g]:
                    st = xspool.tile([128, SUPER * NF], f32, tag="xsuper")
                    nc.sync.dma_start(
                        st[:].rearrange("p (k f) -> p k f", k=SUPER), x_view[sc]
                    )
                    super_tiles[sc] = st

                # --- e1 = tanh(ea @ W1 + b1), feature-major ---
                ea_sb = eapool.tile([128, 128], f32, tag="ea")
                nc.sync.dma_start(ea_sb[:], ea_view[g])
                eaT_ps = ps_eaT.tile([IF, GROUP], f32, tag="eaT")
                for t in range(4):
                    nc.tensor.transpose(
                        eaT_ps[:, 128 * t : 128 * (t + 1)],
                        ea_sb[:, 32 * t : 32 * (t + 1)],
                        ident[:],
                    )
                eaT_sb = eapool.tile([IF, GROUP], f32r, tag="eaT_sb")
                nc.vector.tensor_copy(eaT_sb[:], eaT_ps[:])
                e_ps = ps_e.tile([128, GROUP], f32, tag="e")
                nc.tensor.matmul(
                    e_ps[:], lhsT=w1_sb[:], rhs=eaT_sb[:], start=True, stop=True
                )
                eT_sb = fpool.tile([128, GROUP], f32r, tag="eT")
                nc.scalar.activation(eT_sb[:], e_ps[:], Tanh, bias=b1_sb[:, :1])

                # --- xr via selection expansion of the sorted row stream ---
                rmod_sb = selpool.tile([1, GROUP], bf16, tag="rmod")
                nc.sync.dma_start(
                    rmod_sb[:], rmod_d[0:1, GROUP * g : GROUP * (g + 1)]
                )
                bc_ps = ps_bc.tile([128, GROUP], f32, tag="bc")
                nc.tensor.matmul(
                    bc_ps[:], lhsT=ones_sb[:], rhs=rmod_sb[:], start=True, stop=True
                )
                sel_sb = selpool.tile([128, GROUP], f32, tag="sel")
                nc.vector.tensor_tensor(
                    out=sel_sb[:],
                    in0=bc_ps[:],
                    in1=lane_sb[:].to_broadcast([128, GROUP]),
                    op=mybir.AluOpType.is_equal,
                )
                xrT_ps = ps_x.tile([128, GROUP], f32, tag="xrT")
                for chunk, a, b in segments[g]:
                    sc, kk = divmod(chunk, SUPER)
                    nc.tensor.matmul(
                        xrT_ps[:, a:b],
                        lhsT=super_tiles[sc][:, NF * kk : NF * (kk + 1)],
                        rhs=sel_sb[:, a:b],
                        start=True,
                        stop=True,
                    )
                xrT_sb = fpool.tile([128, GROUP], f32r, tag="xrT_sb")
                nc.vector.tensor_copy(xrT_sb[:], xrT_ps[:])

                # --- xc via indirect gather + PE transpose ---
                xc_sb = gpool.tile([128, GROUP], f32, tag="xc")
                for c in range(4):
                    B = 4 * g + c
                    nc.gpsimd.indirect_dma_start(
                        out=xc_sb[:, 128 * c : 128 * (c + 1)],
                        out_offset=None,
                        in_=x_d[:, :],
                        in_offset=bass.IndirectOffsetOnAxis(
                            ap=cidx_sb[:, B : B + 1], axis=0
                        ),
                    )
                xcT_ps = ps_x.tile([128, GROUP], f32, tag="xcT")
                for c in range(4):
                    nc.tensor.transpose(
                        xcT_ps[:, 128 * c : 128 * (c + 1)],
                        xc_sb[:, 128 * c : 128 * (c + 1)],
                        ident[:],
                    )
                xcT_sb = fpool.tile([128, GROUP], f32r, tag="xcT_sb")
                nc.vector.tensor_copy(xcT_sb[:], xcT_ps[:])

                # --- h = relu(xr@W2a' + xc@W2b' + e1@W2c' + b2') ---
                h_ps = ps_h.tile([128, GROUP], f32, tag="h")
                nc.tensor.matmul(
                    h_ps[:], lhsT=w2a_sb[:], rhs=xrT_sb[:], start=True, stop=False
                )
                nc.tensor.matmul(
                    h_ps[:], lhsT=w2b_sb[:], rhs=xcT_sb[:], start=False, stop=False
                )
                nc.tensor.matmul(
                    h_ps[:], lhsT=w2c_sb[:], rhs=eT_sb[:], start=False, stop=True
                )
                hT_sb = fpool.tile([128, GROUP], f32r, tag="hT")
                nc.scalar.activation(hT_sb[:], h_ps[:], Relu, bias=b2_sb[:, :1])

                # --- y = relu(h @ W3 + b3) ---
                y_ps = ps_y.tile([128, GROUP], f32, tag="y")
                nc.tensor.matmul(
                    y_ps[:], lhsT=w3_sb[:], rhs=hT_sb[:], start=True, stop=True
                )
                yT_sb = opool.tile([128, GROUP], f32, tag="yT")
                nc.vector.tensor_scalar(
                    yT_sb[:],
                    y_ps[:],
                    b3_sb[:, :1],
                    0.0,
                    op0=mybir.AluOpType.add,
                    op1=mybir.AluOpType.max,
                )
                nc.sync.dma_start(yt_d[:, GROUP * g : GROUP * (g + 1)], yT_sb[:])

    nc.compile()
    return nc


def _fold_weights(W1, b1, W2, b2, bn_gamma, bn_beta, bn_mean, bn_var, W3, b3):
    s = np.asarray(bn_gamma, np.float32) / np.sqrt(
        np.asarray(bn_var, np.float32) + BN_EPS
    )
    W2f = (np.asarray(W2, np.float32) * s[None, :]).astype(np.float32)
    b2f = (
        (np.asarray(b2, np.float32) - np.asarray(bn_mean, np.float32)) * s
        + np.asarray(bn_beta, np.float32)
    ).astype(np.float32)
    return (
        np.ascontiguousarray(np.asarray(W1, np.float32)),
        np.ascontiguousarray(W2f[:NF]),
        np.ascontiguousarray(W2f[NF : 2 * NF]),
        np.ascontiguousarray(W2f[2 * NF :]),
        np.ascontiguousarray(np.asarray(W3, np.float32)),
        np.asarray(b1, np.float32).reshape(128, 1).copy(),
        b2f.reshape(128, 1).copy(),
        np.asarray(b3, np.float32).reshape(128, 1).copy(),
    )


def _plan_core(r, cl, ea_part):
    """Row-sort one core's edges; build slot arrays + segment structure."""
    n = r.shape[0]
    n_groups = max(1, -(-n // GROUP))
    ep = n_groups * GROUP
    rows = np.full(ep, N_NODES - 1, np.int64)
    cols = np.zeros(ep, np.int64)
    ea_slot = np.zeros((ep, IF), np.float32)

    order = np.argsort(r, kind="stable")
    slot_of_edge = np.empty(n, np.int64)
    slot_of_edge[order] = np.arange(n)
    rows[:n] = r[order]
    cols[:n] = cl[order]
    ea_slot[:n] = ea_part[order]

    chunks = rows // 128
    segments = []
    super_first_use = []
    seen = set()
    for g in range(n_groups):
        cg = chunks[GROUP * g : GROUP * (g + 1)]
        segs = []
        start = 0
        for i in range(1, GROUP + 1):
            if i == GROUP or cg[i] != cg[start]:
                segs.append((int(cg[start]), start, i))
                start = i
        segments.append(segs)
        need = []
        for ch, _, _ in segs:
            sc = ch // SUPER
            if sc not in seen:
                seen.add(sc)
                need.append(sc)
        super_first_use.append(need)

    # cidx[p, B] = col of slot 512*(B//4) + 128*(B%4) + p
    cidx = cols.reshape(n_groups, 4, 128).transpose(2, 0, 1).reshape(128, -1)
    cidx = np.ascontiguousarray(cidx.astype(np.int32))
    import ml_dtypes
    rmod = np.ascontiguousarray(
        (rows % 128).astype(ml_dtypes.bfloat16).reshape(1, ep)
    )
    ea_dev = (
        ea_slot.reshape(n_groups, 4, 128, IF).transpose(0, 2, 1, 3).reshape(ep, IF)
    )
    return dict(
        n_groups=n_groups,
        segments=segments,
        super_first_use=super_first_use,
        cidx=cidx,
        rmod=rmod,
        ea=np.ascontiguousarray(ea_dev),
        slot_of_edge=slot_of_edge,
    )


def _prepare(inputs):
    x = np.asarray(inputs["x"], np.float32)
    xpad = np.zeros((X_PAD_ROWS, NF), np.float32)
    xpad[:N_NODES] = x
    edge_index = np.asarray(inputs["edge_index"])
    ea = np.asarray(inputs["edge_attr"], np.float32)
    w1, w2a, w2b, w2c, w3, b1t, b2t, b3t = _fold_weights(
        inputs["W1"], inputs["b1"], inputs["W2"], inputs["b2"],
        inputs["bn_gamma"], inputs["bn_beta"], inputs["bn_mean"],
        inputs["bn_var"], inputs["W3"], inputs["b3"],
    )
    E = edge_index.shape[1]
    row = np.asarray(edge_index[0], np.int64)
    col = np.asarray(edge_index[1], np.int64)
    lane = np.arange(128, dtype=np.float32).reshape(128, 1)

    shared = dict(
        x=xpad, w1=w1, w2a=w2a, w2b=w2b, w2c=w2c, w3=w3,
        b1=b1t, b2=b2t, b3=b3t, lane=lane,
    )
    plans, in_maps = [], []
    for c in range(NC):
        lo = min(c * E_PER_CORE, E)
        hi = min(lo + E_PER_CORE, E)
        plan = _plan_core(row[lo:hi], col[lo:hi], ea[lo:hi])
        plans.append(plan)
        in_maps.append(
            dict(shared, ea=plan["ea"], cidx=plan["cidx"], rmod=plan["rmod"])
        )
    return plans, in_maps, E


def _get_programs(plans):
    ncs = []
    for plan in plans:
        key = (
            plan["n_groups"],
            tuple(tuple(s) for segs in plan["segments"] for s in segs),
            tuple(tuple(u) for u in plan["super_first_use"]),
        )
        if key not in _PROGRAM_CACHE:
            _PROGRAM_CACHE[key] = _build_core_program(
                plan["n_groups"], plan["segments"], plan["super_first_use"]
            )
        ncs.append(_PROGRAM_CACHE[key])
    return ncs


def _run_many(ncs, in_maps):
    """Dispatch one program per device asynchronously; fetch all outputs."""
    import jax

    import concourse.mybir as mybir
    from concourse import bass2jax

    bass2jax.install_neuronx_cc_hook()
    devices = jax.devices()[: len(ncs)]

    launched = []
    for c, (nc_c, im) in enumerate(zip(ncs, in_maps)):
        in_names, out_names, out_avals, zero_outs = [], [], [], []
        for alloc in nc_c.m.functions[0].allocations:
            if not isinstance(alloc, mybir.MemoryLocationSet):
                continue
            name = alloc.memorylocations[0].name
            if alloc.kind == "ExternalInput":
                in_names.append(name)
            elif alloc.kind == "ExternalOutput":
                out_names.append(name)
                shape = tuple(alloc.tensor_shape)
                dtype = mybir.dt.np(alloc.dtype)
                out_avals.append(jax.core.ShapedArray(shape, dtype))
                zero_outs.append(np.zeros(shape, dtype))
        n_params = len(in_names)
        all_in_names = tuple(in_names) + tuple(out_names)
        donate = tuple(range(n_params, n_params + len(out_names)))

        def make_body(nc_c, out_avals, all_in_names, out_names):
            def _body(*args):
                outs = bass2jax._bass_exec_p.bind(
                    *args,
                    out_avals=tuple(out_avals),
                    in_names=all_in_names,
                    out_names=tuple(out_names),
                    lowering_input_output_aliases=(),
                    sim_require_finite=True,
                    sim_require_nnan=True,
                    nc=nc_c,
                )
                return tuple(outs)

            return _body

        dev = devices[c]
        pid_name = (
            nc_c.partition_id_tensor.name if nc_c.partition_id_tensor else None
        )
        feeds = dict(im)
        if pid_name is not None:
            feeds[pid_name] = np.array([[c]], np.uint32)
        args = [jax.device_put(np.asarray(feeds[n]), dev) for n in in_names]
        zeros = [jax.device_put(z, dev) for z in zero_outs]
        fn = jax.jit(
            make_body(nc_c, out_avals, all_in_names, out_names),
            donate_argnums=donate,
            keep_unused=True,
        )
        out_arrs = fn(*args, *zeros)
        launched.append((out_names, out_arrs))

    results = []
    for out_names, out_arrs in launched:
        results.append(
            {name: np.asarray(a) for name, a in zip(out_names, out_arrs)}
        )
    return results


def _postprocess(results, plans, E):
    out = np.empty((E, OF), np.float32)
    for c in range(NC):
        lo = min(c * E_PER_CORE, E)
        hi = min(lo + E_PER_CORE, E)
        if hi == lo:
            continue
        y_slot = results[c]["yt"].T  # [EP, 128] in slot order
        out[lo:hi] = y_slot[plans[c]["slot_of_edge"]]
    return out


def kernel(**inputs):
    plans, in_maps, E = _prepare(inputs)
    ncs = _get_programs(plans)
    results = _run_many(ncs, in_maps)
    return _postprocess(results, plans, E)
